# revision 33
# baseline (speedup 1.0000x reference)
"""Fused causal MHA block (QKV proj + 16-head attention + out proj) on 8 trn2 cores.

Sharding: core = (batch b in 0..3, head-group hg in 0..1); each core handles one
batch and 8 heads (512 of 1024 attention features). Host pre-transposes inputs to
feature-major layouts so the device kernel needs no transposes.

v3 (single-exp attention): pass1 computes only the per-row max (mask+max fused in
one DVE tensor_tensor_reduce on the diagonal block); the max is folded into an
augmented 66-row QK^T matmul in pass2, whose exp output P is UNNORMALIZED. The
softmax denominator falls out of the P@V matmul for free via a ones-column
appended to V (M=128 keeps every partition base aligned; even head feats rows
0..63 + denom row 64, odd head denom row 63 + feats rows 64..127). A tail step
computes 1/denom via Ln+Exp (same activation-table set as the softmax exp, so
zero table switches), GpSimd broadcasts it across partitions, and one in-place
fp16 multiply per feature tile normalizes the attention output before the out
projection. fp16 activations/weights end-to-end.
"""
import sys
sys.path.insert(0, "/opt/trn_rl_repo")
import numpy as np

B, N, D = 4, 2048, 1024
H, DH = 16, 64
NCORES = 8
NEG = -60000.0  # fp16-representable; -60000 + score << 0 so exp -> 0

_cache = {}


def _patch_act_tables():
    """Make Exp/Ln resolve only to natural_log_exp_and_others so bacc's
    table-load pass emits one ACT_TABLE_LOAD instead of thrashing."""
    import concourse.bacc as bacc_mod
    import concourse.mybir as mybir
    from concourse.hw_specs import get_activation_tables as orig

    def patched(arch):
        EXP = mybir.ActivationFunctionType.Exp
        LN = mybir.ActivationFunctionType.Ln
        out = {}
        for name, fns in orig(arch).items():
            fns = set(fns)
            if name != "natural_log_exp_and_others":
                fns.discard(EXP)
                fns.discard(LN)
            out[name] = fns
        return out

    bacc_mod.get_activation_tables = patched


def _build(n=N, d3=3 * D, fpc=512, nheads=8, dout=D):
    import concourse.bass as bass
    import concourse.tile as tile
    from concourse import bacc, mybir, masks
    from contextlib import ExitStack

    _patch_act_tables()

    f32, f16 = mybir.dt.float32, mybir.dt.float16
    AX, ALU, ACT = mybir.AxisListType, mybir.AluOpType, mybir.ActivationFunctionType

    nt = n // 128           # seq tiles
    nsc = n // 512          # seq chunks
    nk = d3 // 128          # k chunks total
    nkh = nk // 2           # k chunks per half pass
    nftqk = 2 * fpc // 128  # q+k feature tiles
    nfto = fpc // 128       # attn-out feature tiles
    npair = nheads // 2
    noc = dout // 512       # out-proj col chunks

    nc = bacc.Bacc("TRN2", target_bir_lowering=False, debug=False,
                   num_devices=NCORES)
    xT = nc.dram_tensor("xT", [d3, n], f16, kind="ExternalInput").ap()
    wqkT = nc.dram_tensor("wqkT", [d3, 2 * fpc], f16, kind="ExternalInput").ap()
    wvT = nc.dram_tensor("wvT", [d3, fpc], f16, kind="ExternalInput").ap()
    bqk = nc.dram_tensor("bqk", [128, nftqk], f32, kind="ExternalInput").ap()
    bv = nc.dram_tensor("bv", [128, fpc], f32, kind="ExternalInput").ap()
    woT = nc.dram_tensor("woT", [fpc, dout], f16, kind="ExternalInput").ap()
    bo = nc.dram_tensor("bo", [128, dout], f32, kind="ExternalInput").ap()
    mask1 = nc.dram_tensor("mask1", [128, 128], f16, kind="ExternalInput").ap()
    maskT = nc.dram_tensor("maskT", [128, 128], f16, kind="ExternalInput").ap()
    neg1 = nc.dram_tensor("neg1", [2, n], f16, kind="ExternalInput").ap()
    out = nc.dram_tensor("out", [n, dout], f32, kind="ExternalOutput").ap()

    with tile.TileContext(nc) as tc, ExitStack() as ctx:
        const = ctx.enter_context(tc.tile_pool(name="const", bufs=1))
        resid = ctx.enter_context(tc.tile_pool(name="resid", bufs=1))
        wftp = ctx.enter_context(tc.tile_pool(name="wft", bufs=3))
        wvp = ctx.enter_context(tc.tile_pool(name="wv", bufs=3))
        xtp = ctx.enter_context(tc.tile_pool(name="xt", bufs=2))
        augp = ctx.enter_context(tc.tile_pool(name="aug", bufs=2))
        ptp = ctx.enter_context(tc.tile_pool(name="pt", bufs=3))
        stat = ctx.enter_context(tc.tile_pool(name="stat", bufs=4))
        outp = ctx.enter_context(tc.tile_pool(name="outs", bufs=2))
        sps = ctx.enter_context(tc.tile_pool(name="sps", bufs=6, space="PSUM"))
        pvps = ctx.enter_context(tc.tile_pool(name="pvps", bufs=2, space="PSUM"))

        ident = const.tile([128, 128], f32, tag="ident")
        masks.make_identity(nc, ident[:])
        identh = const.tile([128, 128], f16, tag="identh")
        masks.make_identity(nc, identh[:])
        m1 = const.tile([128, 128], f16, tag="m1")
        nc.sync.dma_start(m1[:], mask1)
        mTt = const.tile([128, 128], f16, tag="mT")
        nc.sync.dma_start(mTt[:], maskT)
        bqk_t = const.tile([128, nftqk], f32, tag="bqk")
        nc.sync.dma_start(bqk_t[:], bqk)
        bv_t = const.tile([128, fpc], f32, tag="bv")
        nc.sync.dma_start(bv_t[:], bv)
        bo_t = const.tile([128, dout], f32, tag="bo")
        nc.sync.dma_start(bo_t[:], bo)
        woT_t = const.tile([128, nfto, dout], f16, tag="woT")
        for ft in range(nfto):
            nc.sync.dma_start(woT_t[:, ft, :], woT[128 * ft:128 * ft + 128, :])

        qkT = resid.tile([128, nftqk, n], f16, tag="qkT")   # ft 0..3 q, 4..7 k
        # v, pair-major blocks of 192 cols:
        #   [even feats 0..63 | ones 64 | zeros 65..127 | odd feats 128..191]
        # even head lhsT window = cols 0..127 -> denom lands on psum row 64;
        # odd head lhsT window = cols 64..191 -> denom lands on psum row 0.
        # Both rows are 32-aligned, which PSUM reads require.
        vv = resid.tile([128, nt, npair, 192], f16, tag="vv")
        aoT = resid.tile([128, nfto, n], f16, tag="aoT")    # attn out, feat-major
        # ln(softmax denom), written by ACT straight from PSUM. Row 64 holds
        # even heads, row 0 odd heads; free slot = 4*(h//2) + c.
        lnd = resid.tile([128, 4 * npair, 512], f32, tag="lnd")
        rcp = resid.tile([128, 4 * npair, 512], f16, tag="rcp")

        nc.gpsimd.memset(vv[:, :, :, 64:128], 0.0)
        nc.gpsimd.memset(vv[:, :, :, 64:65], 1.0)
        nc.gpsimd.memset(lnd[:], 0.0)  # unused slots stay finite for batch Exp
        ones_t = const.tile([128, 64], f16, tag="ones")
        nc.gpsimd.memset(ones_t[:], 1.0)

        # ---- Phase 1: QKV projection (two half-K passes) ----
        # ft order puts head-0's q (ft 0) and k (ft 4) features first so
        # pass1 of head 0 can interleave with the tail of the last chunk.
        FT_ORDER = [0, 4, 1, 5, 2, 6, 3, 7]

        def phase1_gen():
            for sc in range(nsc):
                pss = []
                for _pi in range(4):
                    vps = sps.tile([128, fpc], f32, tag="s")
                    pss.append(vps)
                for half in range(2):
                    k0 = nkh * half
                    xt = xtp.tile([128, nkh, 512], f16, tag="xt")
                    nc.sync.dma_start(
                        xt[:], xT[128 * k0:128 * (k0 + nkh),
                                  512 * sc:512 * sc + 512].rearrange(
                            "(c p) m -> p c m", p=128))
                    for ft in FT_ORDER:
                        wf = wftp.tile([128, nkh, 128], f16, tag="wf")
                        nc.sync.dma_start(
                            wf[:], wqkT[128 * k0:128 * (k0 + nkh),
                                        128 * ft:128 * ft + 128].rearrange(
                                "(c p) m -> p c m", p=128))
                        ps = pvps.tile([128, 512], f32, tag="pv")
                        for k in range(nkh):
                            nc.tensor.matmul(ps[:], wf[:, k, :], xt[:, k, :],
                                             start=(k == 0),
                                             stop=(k == nkh - 1))
                        dst = qkT[:, ft, 512 * sc:512 * sc + 512]
                        if half == 0:
                            # bias-add on the otherwise-idle ACT engine
                            nc.scalar.activation(dst, ps[:], ACT.Identity,
                                                 bias=bqk_t[:, ft:ft + 1])
                        else:
                            nc.vector.tensor_add(dst, ps[:], dst)
                        if sc == nsc - 1 and half == 1 and ft == 4:
                            yield  # head 0's q/k complete: start its pass1
                    for k in range(nkh):
                        wv_t = wvp.tile([128, fpc], f16, tag="wv")
                        nc.sync.dma_start(
                            wv_t[:], wvT[128 * (k0 + k):128 * (k0 + k + 1), :])
                        for ss in range(4):
                            nc.tensor.matmul(
                                pss[ss][:], xt[:, k, 128 * ss:128 * ss + 128],
                                wv_t[:],
                                start=(half == 0 and k == 0),
                                stop=(half == 1 and k == nkh - 1))
                        if sc == nsc - 1 and half == 1:
                            yield
                for ss in range(4):
                    src = pss[ss][:].rearrange("p (a b f) -> p a b f",
                                               b=2, f=64)
                    bsr = bv_t[:].rearrange("p (a b f) -> p a b f", b=2, f=64)
                    for par in range(2):
                        nc.vector.tensor_add(
                            vv[:, 4 * sc + ss, :, 128 * par:128 * par + 64],
                            src[:, :, par, :], bsr[:, :, par, :])

        # ---- Phase 2: attention ----
        # pass1(h+1) and pass2(h) are emitted as generators and interleaved
        # step-by-step so the per-engine FIFO queues alternate between the
        # two instruction streams: pass2's dependency stalls (st->exp->PV)
        # are filled by always-ready pass1 matmuls, keeping the PE dense
        # enough that HAM never re-throttles the clock.
        aug_tiles = {}

        def pass1_gen(h):
            pb = 64 * (h % 2)
            ftq, ftk = h // 2, nftqk // 2 + h // 2
            augcols = stat.tile([128, nt], f32, tag="augcols")
            for i in range(nt):
                nchunks = i // 4 + 1
                rmx = stat.tile([128, 4], f32, tag="rmx")
                for jj in range(nchunks):
                    W = 512 if jj < i // 4 else 128 * (i % 4) + 128
                    ps = sps.tile([128, 512], f32, tag="s")
                    diag = jj == nchunks - 1
                    if diag:
                        # causal mask via PE: write mask into the diagonal
                        # block, then accumulate scores per-element on top
                        nc.tensor.matmul(ps[:, W - 128:W], identh[:], m1[:],
                                         start=True, stop=False)
                    nc.tensor.matmul(
                        ps[:, :W],
                        qkT[pb:pb + 64, ftq, 128 * i:128 * i + 128],
                        qkT[pb:pb + 64, ftk, 512 * jj:512 * jj + W],
                        start=not diag, stop=True)
                    nc.vector.tensor_reduce(rmx[:, jj:jj + 1], ps[:, :W],
                                            AX.X, ALU.max)
                    yield
                nc.vector.tensor_reduce(augcols[:, i:i + 1],
                                        rmx[:, :nchunks], AX.X, ALU.max)
            achl = stat.tile([128, 2 * nt], f32, tag="achl")
            MAGIC = 12582912.0  # 1.5 * 2**23: rounds |x|<2^22 to nearest int
            nc.vector.tensor_scalar(achl[:, 0:nt], augcols[:], 2.0, MAGIC,
                                    ALU.mult, ALU.add)
            nc.vector.tensor_scalar(achl[:, 0:nt], achl[:, 0:nt], MAGIC, 0.5,
                                    ALU.subtract, ALU.mult)
            nc.vector.tensor_sub(achl[:, nt:2 * nt], augcols[:], achl[:, 0:nt])
            tp = sps.tile([2 * nt, 128], f32, tag="s")
            nc.tensor.transpose(tp[:], achl[:], ident[:])
            trow = stat.tile([2 * nt, 128], f16, tag="trow")
            nc.vector.tensor_copy(trow[:], tp[:])
            qaug = augp.tile([66, n], f16, tag="qaug")
            kaug = augp.tile([66, n], f16, tag="kaug")
            nc.sync.dma_start(qaug[0:64, :], qkT[pb:pb + 64, ftq, :])
            nc.sync.dma_start(kaug[0:64, :], qkT[pb:pb + 64, ftk, :])
            nc.sync.dma_start(
                qaug[64:66, :].rearrange("o (t f) -> o t f", f=128), trow[:])
            nc.sync.dma_start(kaug[64:66, :], neg1)
            aug_tiles[h] = (qaug, kaug)

        def pass2_gen(h):
            pb = 64 * (h % 2)
            ftq = h // 2
            pr, par = h // 2, h % 2
            qaug, kaug = aug_tiles.pop(h)
            for c in range(nsc):
                pv = pvps.tile([128, 512], f32, tag="pv")
                nj = 4 * c + 4
                pend = []
                for j in range(nj + 2):
                    if j < nj:
                        qs = max(512 * c, 128 * j)
                        W = 512 * (c + 1) - qs
                        st = sps.tile([128, 512], f32, tag="s")
                        diag = j >= 4 * c
                        if diag:
                            nc.tensor.matmul(st[:, 0:128], identh[:], mTt[:],
                                             start=True, stop=False)
                        nc.tensor.matmul(st[:, :W],
                                         kaug[:, 128 * j:128 * j + 128],
                                         qaug[:, qs:qs + W],
                                         start=not diag, stop=True)
                        pt = ptp.tile([128, 512], f16, tag="pt")
                        nc.scalar.activation(pt[:, :W], st[:, :W], ACT.Exp,
                                             scale=8.0)
                        pend.append((j, pt, qs, W))
                    if j >= 2:
                        jj, pt, qs, W = pend[j - 2]
                        o0 = qs - 512 * c
                        nc.tensor.matmul(
                            pv[:, o0:o0 + W],
                            vv[:, jj, pr, 64 * par:64 * par + 128], pt[:, :W],
                            start=(jj == 0), stop=(jj == nj - 1))
                    yield
                nc.scalar.activation(
                    aoT[pb:pb + 64, ftq, 512 * c:512 * c + 512],
                    pv[pb:pb + 64, :], ACT.Copy)
                dr = 64 * (1 - par)  # denom row: 64 for even heads, 0 for odd
                nc.scalar.activation(lnd[dr:dr + 1, 4 * pr + c, :],
                                     pv[dr:dr + 1, :], ACT.Ln)
                yield

        def norm_gen(fts):
            # 1/denom = exp(-ln(denom)); per-ft Exp batches so ft f unlocks
            # as soon as heads 2f and 2f+1 finish, broadcast via K=1 matmul
            # against a ones row, one in-place multiply per 512-col chunk.
            for ft in fts:
                for dr in (0, 64):
                    nc.scalar.activation(rcp[dr:dr + 1, 4 * ft:4 * ft + 4, :],
                                         lnd[dr:dr + 1, 4 * ft:4 * ft + 4, :],
                                         ACT.Exp, scale=-1.0)
                for c in range(nsc):
                    bc_ps = sps.tile([128, 512], f32, tag="s")
                    nc.tensor.matmul(bc_ps[0:64, :], ones_t[64:65, 0:64],
                                     rcp[64:65, 4 * ft + c, :],
                                     start=True, stop=True)
                    nc.tensor.matmul(bc_ps[64:128, :], ones_t[0:1, 0:64],
                                     rcp[0:1, 4 * ft + c, :],
                                     start=True, stop=True)
                    nc.vector.tensor_mul(aoT[:, ft, 512 * c:512 * c + 512],
                                         aoT[:, ft, 512 * c:512 * c + 512],
                                         bc_ps[:])
                    yield

        def drive(*gens):
            live = [g for g in gens if g is not None]
            while live:
                for g in list(live):
                    try:
                        next(g)
                    except StopIteration:
                        live.remove(g)

        drive(phase1_gen(), pass1_gen(0))
        for h in range(1, nheads):
            drive(pass2_gen(h - 1), pass1_gen(h))
        # fts 0..2 only need heads <= 5, so they can fill pass2(7)'s stalls;
        # ft 3 needs head 7's denominators and must come after.
        drive(pass2_gen(nheads - 1), norm_gen(range(nfto - 1)))
        drive(norm_gen([nfto - 1]))

        # ---- Phase 3: output projection ----
        for s in range(nt):
            for oc in range(noc):
                ps = sps.tile([128, 512], f32, tag="s")
                for ft in range(nfto):
                    nc.tensor.matmul(ps[:], aoT[:, ft, 128 * s:128 * s + 128],
                                     woT_t[:, ft, 512 * oc:512 * oc + 512],
                                     start=(ft == 0), stop=(ft == nfto - 1))
                ot = outp.tile([128, 512], f32, tag="ot")
                nc.vector.tensor_add(ot[:], ps[:],
                                     bo_t[:, 512 * oc:512 * oc + 512])
                nc.sync.dma_start(out[128 * s:128 * s + 128,
                                      512 * oc:512 * oc + 512], ot[:])

    nc.compile()
    return nc


def _in_maps(q, k, v, w_qkv, b_qkv, w_out, b_out):
    x = np.concatenate([q, k, v], axis=-1)  # (B, N, 3D)
    tri = np.triu(np.full((128, 128), NEG, np.float32), 1)  # 0 on/below diag
    maps = []
    for core in range(NCORES):
        b, hg = core // 2, core % 2
        fs = slice(512 * hg, 512 * hg + 512)
        wq = w_qkv[0 * D:1 * D][fs]
        wk = w_qkv[1 * D:2 * D][fs]
        wv = w_qkv[2 * D:3 * D][fs]
        bq = b_qkv[0 * D:1 * D][fs]
        bk = b_qkv[1 * D:2 * D][fs]
        bvb = b_qkv[2 * D:3 * D][fs]
        maps.append({
            "xT": np.ascontiguousarray(x[b].T).astype(np.float16),
            "wqkT": np.ascontiguousarray(
                np.concatenate([wq, wk], 0).T).astype(np.float16),
            "wvT": np.ascontiguousarray(wv.T).astype(np.float16),
            "bqk": np.ascontiguousarray(
                np.concatenate([bq, bk]).reshape(8, 128).T),
            "bv": np.tile(bvb[None, :], (128, 1)),
            "woT": np.ascontiguousarray(w_out[:, fs].T).astype(np.float16),
            "bo": np.tile(b_out[None, :], (128, 1)) if hg == 0
                  else np.zeros((128, D), np.float32),
            "mask1": tri.astype(np.float16),
            "maskT": np.ascontiguousarray(tri.T).astype(np.float16),
            "neg1": -np.ones((2, N), np.float16),
        })
    return maps


def kernel(q, k, v, w_qkv, b_qkv, w_out, b_out, _trace=False):
    from concourse import bass_utils
    if "nc" not in _cache:
        _cache["nc"] = _build()
    nc = _cache["nc"]
    maps = _in_maps(np.asarray(q, np.float32), np.asarray(k, np.float32),
                    np.asarray(v, np.float32), np.asarray(w_qkv, np.float32),
                    np.asarray(b_qkv, np.float32), np.asarray(w_out, np.float32),
                    np.asarray(b_out, np.float32))
    res = bass_utils.run_bass_kernel_spmd(nc, maps, core_ids=list(range(NCORES)),
                                          trace=_trace)
    outs = [np.asarray(res.results[c]["out"], np.float32) for c in range(NCORES)]
    full = np.stack([outs[2 * b] + outs[2 * b + 1] for b in range(B)], 0)
    if _trace:
        return full, res
    return full
